# revision 1
# baseline (speedup 1.0000x reference)
"""CapsuleLayer (dynamic routing, 3 iterations) Trainium2 Bass kernel.

Full inputs:  input_vectors [32, 2048, 16] f32, weight_matrix [1, 64, 32, 16] f32
Full output:  [32, 64, 32] f32

Sharding: data-parallel over batch; each of 8 NeuronCores processes 4 batches.
weight-derived constants are replicated. No collectives.

Algorithm restructuring (never materializes u = [B,N,O,D] = 537MB):
  xs       = squash(x)                       (per-row scale g = n2/((eps+n2)(1e-8+n)))
  iter 0:  c uniform -> t0[o,i] = (1/64) sum_n xs[n,i]        (ones matmul)
  iter k:  logits = xs @ wv_sum.T            (bf16 matmul, K=16, row-tiled)
           e = exp(logits); Z = sum_o e; xz = xs / Z
           t[o,i] = sum_n e[n,o] * xz[n,i]   (f32 matmul, K=128, col-tiled by batch)
  wv      = h * (M2 @ t),  M2 = W^T W (host-precomputed Gram),  h = squash scale of s
            (uses n2 = ||s||^2 = t . (M2 @ t) so s itself is only built at the end)
  output  v = h * (W @ t)  at the last iteration.
Iteration 2 logits use rhs wv0+wv1 (linearity) so no cross-iteration PSUM state.
"""

import os

os.environ.setdefault("MYCRO_LOCAL_CACHE", "1")

import numpy as np
import ml_dtypes

import concourse.bass as bass
import concourse.tile as tile
from concourse import bacc, mybir
from concourse.bass_utils import run_bass_kernel_spmd

AF = mybir.ActivationFunctionType
ALU = mybir.AluOpType
F32 = mybir.dt.float32
BF16 = mybir.dt.bfloat16

N_CORES = 8
B = 4          # batches per core
N = 2048       # input capsules
O = 64         # output capsules
DI = 16        # input capsule dim
D = 32         # output capsule dim
G = 16         # n-groups of 128 per batch
EPS = 0.5

# wvT transpose fallback: replicated-weights AP (step-0) single transpose vs 4.
# (walrus birverifier rejects multi-free-dim weights APs, so keep False)
SINGLE_TRANSPOSE = False

# debug bisect: 0=loads+squash only, 1=+xsT transposes, 2=+iter0, 3=+iter1, 9=full
DEBUG_LEVEL = int(os.environ.get("CAPS_DEBUG_LEVEL", "9"))


def _strip(b, g):
    """(row_base, col_base) of the xsT strip for (batch b, n-group g).

    Quad layout: the 4 concurrent K=16 agreement matmuls of a quad sit at row
    groups 0/32/64/96 = (b%2)*64 + (g//8)*32 and write logits cols g*64 which
    lands groups g and g+8 in different PSUM banks.
    """
    r = (b % 2) * 64 + (g // 8) * 32
    c = ((b // 2) * 8 + (g % 8)) * 128
    return r, c


def build_kernel(nc: bass.Bass, tc: tile.TileContext):
    from contextlib import ExitStack
    ctx = ExitStack()
    x = nc.dram_tensor("x", [B, N, DI], F32, kind="ExternalInput").ap()
    wrep = nc.dram_tensor("wrep", [128, D * DI], F32, kind="ExternalInput").ap()
    m2rep = nc.dram_tensor("m2rep", [128, DI * DI], F32, kind="ExternalInput").ap()
    ident = nc.dram_tensor("ident", [128, 128], BF16, kind="ExternalInput").ap()
    vout = nc.dram_tensor("vout", [B, O, D], F32, kind="ExternalOutput").ap()

    const = ctx.enter_context(tc.tile_pool(name="const", bufs=1))
    big = ctx.enter_context(tc.tile_pool(name="big", bufs=1))
    small = ctx.enter_context(tc.tile_pool(name="small", bufs=2))
    psum = ctx.enter_context(tc.tile_pool(name="psum", bufs=2, space="PSUM"))
    psum1 = ctx.enter_context(tc.tile_pool(name="psum1", bufs=1, space="PSUM"))

    # ---- constants ----
    w_sb = const.tile([128, D * DI], F32, tag="w_sb")
    m2_sb = const.tile([128, DI * DI], F32, tag="m2_sb")
    id_sb = const.tile([128, 128], BF16, tag="id_sb")
    ones64 = const.tile([128, O], F32, tag="ones64")
    nc.sync.dma_start(w_sb[:], wrep)
    nc.sync.dma_start(m2_sb[:], m2rep)
    nc.sync.dma_start(id_sb[:], ident)
    nc.gpsimd.memset(ones64[:], 1.0 / O)

    # ---- load x:  xr [128, (b, g, i)] ----
    xr = big.tile([128, B * G * DI], F32, tag="xr")
    nc.sync.dma_start(
        xr[:].rearrange("p (b g i) -> p b g i", b=B, g=G),
        x.rearrange("b (g p) i -> p b g i", p=128),
    )

    # ---- squash ----
    xsq = big.tile([128, B * G * DI], F32, tag="xsq")
    nc.scalar.square(xsq[:], xr[:])
    n2x = small.tile([128, B * G], F32, tag="n2x")
    nc.vector.reduce_sum(n2x[:], xsq[:].rearrange("p (r i) -> p r i", i=DI), axis=mybir.AxisListType.X)
    nx = small.tile([128, B * G], F32, tag="nx")
    nc.scalar.sqrt(nx[:], n2x[:])
    nc.vector.tensor_scalar_add(nx[:], nx[:], 1e-8)
    denx = small.tile([128, B * G], F32, tag="denx")
    nc.vector.scalar_tensor_tensor(denx[:], n2x[:], 0.5, nx[:], op0=ALU.add, op1=ALU.mult)
    nc.vector.reciprocal(denx[:], denx[:])
    gx = small.tile([128, B * G], F32, tag="gx")
    nc.vector.tensor_mul(gx[:], n2x[:], denx[:])
    xs = big.tile([128, B * G * DI], F32, tag="xs")
    nc.vector.tensor_mul(
        xs[:].rearrange("p (r i) -> p r i", i=DI),
        xr[:].rearrange("p (r i) -> p r i", i=DI),
        gx[:].unsqueeze(2).broadcast_to([128, B * G, DI]),
    )

    # ---- bf16 copy of xs in the padded/permuted layout + DMA transposes -> xsT
    # xsp col = P*1024 + gl*128 + bl*64 + gh*32 + i  (b = 2P+bl, g = gh*8+gl)
    xsp = big.tile([128, 2048], BF16, tag="xsp")
    nc.gpsimd.memset(xsp[:], 0.0)
    xspv = xsp[:].rearrange("p (pp gl bv gh c) -> p pp gl bv gh c", pp=2, gl=8, bv=2, gh=2)
    for P in range(2):
        for bl in range(2):
            b = 2 * P + bl
            nc.vector.tensor_copy(
                xspv[:, P, :, bl, :, :DI],
                xs[:, b * G * DI:(b + 1) * G * DI].rearrange(
                    "p (gh gl i) -> p gl gh i", gh=2, gl=8
                ),
            )
    xsT = big.tile([128, 2048], BF16, tag="xsT")
    if DEBUG_LEVEL >= 1:
        for ch in range(16):
            nc.sync.dma_start(
                xsT[:, ch * 128:(ch + 1) * 128],
                xsp[:, ch * 128:(ch + 1) * 128],
                transpose=True,
            )

    # ---- persistent state ----
    e_sb = big.tile([128, B * G * O], F32, tag="e_sb")
    rz = small.tile([128, B * G], F32, tag="rz")
    xz = big.tile([128, B * G * DI], F32, tag="xz")
    wv0f = [const.tile([128, DI], F32, tag=f"wv0f_{P}", name=f"wv0f_{P}") for P in range(2)]
    trc = [None, None]

    if DEBUG_LEVEL < 2:
        # dump a slice of xs as output and stop
        dbg = small.tile([128, D], F32, tag="dbg")
        nc.vector.tensor_copy(dbg[:], xs[:, :D])
        for P in range(2):
            nc.sync.dma_start(vout[2 * P:2 * P + 2].rearrange("b o d -> (b o) d"), dbg[:])
        ctx.close()
        return

    n_iters = 3 if DEBUG_LEVEL >= 4 else (DEBUG_LEVEL - 1)
    for it in range(3):
        if it >= n_iters and DEBUG_LEVEL < 4:
            # emit output from whatever small-stage state exists
            break
        if it > 0:
            # ---- agreements -> logits (bf16, K=16, 4-way row-tiled quads) ----
            for b in range(B):
                L = psum.tile([128, G * O], F32, tag="logits")
                # gl-major order: consecutive matmuls alternate row-group and
                # PSUM bank (g and g+8 differ in both)
                for g in [gh * 8 + gl for gl in range(8) for gh in range(2)]:
                    r, c = _strip(b, g)
                    nc.tensor.matmul(
                        L[:, g * O:(g + 1) * O],
                        lhsT=xsT[r:r + DI, c:c + 128],
                        rhs=trc[b // 2][r:r + DI, (b % 2) * O:(b % 2 + 1) * O],
                        tile_position=(r, 0),
                        start=True,
                        stop=True,
                    )
                # ---- softmax pieces ----
                eb = e_sb[:, b * G * O:(b + 1) * G * O]
                nc.scalar.activation(eb, L[:, :], AF.Exp)
                zb = small.tile([128, G], F32, tag="zb")
                nc.vector.reduce_sum(
                    zb[:], eb.rearrange("p (g o) -> p g o", o=O), axis=mybir.AxisListType.X
                )
                nc.vector.reciprocal(rz[:, b * G:(b + 1) * G], zb[:])
                nc.vector.tensor_mul(
                    xz[:, b * G * DI:(b + 1) * G * DI].rearrange("p (g i) -> p g i", i=DI),
                    xs[:, b * G * DI:(b + 1) * G * DI].rearrange("p (g i) -> p g i", i=DI),
                    rz[:, b * G:(b + 1) * G].unsqueeze(2).broadcast_to([128, G, DI]),
                )

        for P in range(2):
            # ---- t matmul (f32, K=128, col-tiled by batch pair) ----
            tps = psum.tile([128, DI], F32, tag="tps")
            for g in range(G):
                for bl in range(2):
                    b = 2 * P + bl
                    if it == 0:
                        lhsT = ones64[:, :]
                        rhs = xs[:, (b * G + g) * DI:(b * G + g + 1) * DI]
                    else:
                        lhsT = e_sb[:, (b * G + g) * O:(b * G + g + 1) * O]
                        rhs = xz[:, (b * G + g) * DI:(b * G + g + 1) * DI]
                    nc.tensor.matmul(
                        tps[bl * O:(bl + 1) * O, :],
                        lhsT=lhsT,
                        rhs=rhs,
                        tile_position=(0, bl * O),
                        start=(g == 0),
                        stop=(g == G - 1),
                        skip_group_check=True,
                    )

            # ---- small stage: q, n2, h ----
            # (tensor_tensor_reduce crashes the device on this HW path; use
            # mult + reduce instead, and stage PSUM t -> SBUF via ACT first)
            t_sb = small.tile([128, DI], F32, tag="t_sb")
            nc.scalar.copy(t_sb[:], tps[:])
            n2t = small.tile([128, 1], F32, tag="n2t")
            if it < 2:
                qm = small.tile([128, DI * DI], F32, tag="qm")
                nc.vector.tensor_mul(
                    qm[:].rearrange("p (i j) -> p i j", j=DI),
                    m2_sb[:].rearrange("p (i j) -> p i j", j=DI),
                    t_sb[:].unsqueeze(1).broadcast_to([128, DI, DI]),
                )
                q = small.tile([128, DI], F32, tag="q")
                nc.vector.reduce_sum(
                    q[:], qm[:].rearrange("p (i j) -> p i j", j=DI), axis=mybir.AxisListType.X
                )
                scr = small.tile([128, DI], F32, tag="scr")
                nc.vector.tensor_mul(scr[:], t_sb[:], q[:])
                nc.vector.reduce_sum(
                    n2t[:], scr[:].rearrange("p (u j) -> p u j", u=1), axis=mybir.AxisListType.X
                )
            else:
                sm = small.tile([128, D * DI], F32, tag="sm")
                nc.vector.tensor_mul(
                    sm[:].rearrange("p (d j) -> p d j", j=DI),
                    w_sb[:].rearrange("p (d j) -> p d j", j=DI),
                    t_sb[:].unsqueeze(1).broadcast_to([128, D, DI]),
                )
                s_sb = small.tile([128, D], F32, tag="s_sb")
                nc.vector.reduce_sum(
                    s_sb[:], sm[:].rearrange("p (d j) -> p d j", j=DI), axis=mybir.AxisListType.X
                )
                scr2 = small.tile([128, D], F32, tag="scr2")
                nc.vector.tensor_mul(scr2[:], s_sb[:], s_sb[:])
                nc.vector.reduce_sum(
                    n2t[:], scr2[:].rearrange("p (u d) -> p u d", u=1), axis=mybir.AxisListType.X
                )
            nt = small.tile([128, 1], F32, tag="nt")
            nc.scalar.sqrt(nt[:], n2t[:])
            nc.vector.tensor_scalar_add(nt[:], nt[:], 1e-8)
            dent = small.tile([128, 1], F32, tag="dent")
            nc.vector.scalar_tensor_tensor(dent[:], n2t[:], 0.5, nt[:], op0=ALU.add, op1=ALU.mult)
            nc.vector.reciprocal(dent[:], dent[:])
            h = small.tile([128, 1], F32, tag="h")
            nc.vector.tensor_mul(h[:], n2t[:], dent[:])

            if it < 2:
                # ---- wv (bf16) + replicated transpose -> trc[P] ----
                wv_bf = small.tile([128, 32], BF16, tag="wv_bf")
                nc.gpsimd.memset(wv_bf[:], 0.0)
                if it == 0:
                    nc.vector.tensor_scalar_mul(wv0f[P][:], q[:], h[:])
                    nc.vector.tensor_scalar_mul(wv_bf[:, :DI], q[:], h[:])
                else:
                    nc.vector.scalar_tensor_tensor(
                        wv_bf[:, :DI], q[:], h[:], wv0f[P][:], op0=ALU.mult, op1=ALU.add
                    )
                trp = psum1.tile([128, 128], BF16, tag="trp")
                if SINGLE_TRANSPOSE:
                    nc.tensor.transpose(
                        trp[:],
                        wv_bf[:].unsqueeze(1).broadcast_to([128, 4, 32]),
                        id_sb[:],
                    )
                else:
                    # transpose all 32 cols (pads are zeros) so each writes a
                    # full 32-row strip -> trp fully initialized for the copy
                    for r4 in range(4):
                        nc.tensor.transpose(
                            trp[r4 * 32:(r4 + 1) * 32, :],
                            wv_bf[:, :],
                            id_sb[:],
                            tile_position=(0, r4 * 32),
                        )
                t_sb = small.tile([128, 128], BF16, tag="trc")
                nc.scalar.copy(t_sb[:], trp[:])
                trc[P] = t_sb
            else:
                # ---- output v = h * s ----
                v_sb = small.tile([128, D], F32, tag="v_sb")
                nc.vector.tensor_scalar_mul(v_sb[:], s_sb[:], h[:])
                nc.sync.dma_start(
                    vout[2 * P:2 * P + 2].rearrange("b o d -> (b o) d"),
                    v_sb[:],
                )
    ctx.close()


_CACHE = {}


def _get_module():
    if "nc" not in _CACHE:
        nc = bacc.Bacc("TRN2", target_bir_lowering=False, debug=False,
                       enable_asserts=False, num_devices=N_CORES)
        with tile.TileContext(nc) as tc:
            build_kernel(nc, tc)
        nc.compile()
        _CACHE["nc"] = nc
    return _CACHE["nc"]


def _host_inputs(input_vectors, weight_matrix):
    W0 = np.asarray(weight_matrix, dtype=np.float32)[0]          # [O, D, DI]
    M2 = np.einsum("odi,odj->oij", W0, W0).astype(np.float32)    # [O, DI, DI]
    wrep = np.tile(W0.reshape(O, D * DI), (2, 1)).astype(np.float32)
    m2rep = np.tile(M2.reshape(O, DI * DI), (2, 1)).astype(np.float32)
    ident = np.eye(128, dtype=ml_dtypes.bfloat16)
    x = np.ascontiguousarray(np.asarray(input_vectors, dtype=np.float32))
    in_maps = []
    for c in range(N_CORES):
        in_maps.append({
            "x": np.ascontiguousarray(x[c * B:(c + 1) * B]),
            "wrep": wrep,
            "m2rep": m2rep,
            "ident": ident,
        })
    return in_maps


def run(input_vectors, weight_matrix, trace=False, tmpdir=None):
    nc = _get_module()
    in_maps = _host_inputs(input_vectors, weight_matrix)
    res = run_bass_kernel_spmd(
        nc, in_maps, core_ids=list(range(N_CORES)), trace=trace, tmpdir=tmpdir
    )
    out = np.concatenate([res.results[c]["vout"] for c in range(N_CORES)], axis=0)
    return out.astype(np.float32), res


def kernel(input_vectors, weight_matrix):
    out, _ = run(input_vectors, weight_matrix, trace=False)
    return out



# revision 12
# speedup vs baseline: 1.0338x; 1.0338x over previous
"""CapsuleLayer (dynamic routing, 3 iterations) Trainium2 Bass kernel.

Full inputs:  input_vectors [32, 2048, 16] f32, weight_matrix [1, 64, 32, 16] f32
Full output:  [32, 64, 32] f32

Sharding: data-parallel over batch; each of 8 NeuronCores processes 4 batches.
No collectives.

Restructured for minimal tensor-engine instruction count (the baseline was
issue-bound: ~1056 matmul+ldweights of tiny shapes). All 4 batches are fused
into every matmul:

  xs_bf [128, (g,b,i)]   squashed inputs, bf16, n on partitions (16 strips g)
  xsT4  [64=(b,i), 2048] PE-transposed inputs (stationary for agreements)
  agreements: per strip g ONE matmul: lhsT=xsT4 strip [64,128],
      rhs=blockdiag(wv_b) [64, 256] -> logits4 [128, (b,o)=256].  16 MM/iter.
  softmax: exp on scalar engine in [128,1024] chunks; Z + 1/Z + xz on vector.
  t: per strip ONE matmul: lhsT=xz strip [128,(b,i)=64], rhs=e strip
      [128,(b,o)=256] -> tT [ (b,i)=64, (b,o)=256 ] accumulated in PSUM;
      only the 4 diagonal 16x64 blocks are used (cross-batch blocks are
      computed-but-ignored; streaming cost is identical).  16 MM/iter.
  tT diag blocks -> PE transpose -> t4 [128=(b%2,o), (b//2,i)=32]
  squash(s)/wv on [o]-partition layout via vector ops; wv -> PE transpose ->
      blockdiag rhs for the next iteration's agreements.

sqrt is computed as Exp(0.5*Ln(x)) so the scalar engine only ever uses the
natural_log_exp activation table (no 1.3us ACT_TABLE_LOAD thrash; Sqrt lives
in a different table than Exp).
"""

import os

os.environ.setdefault("MYCRO_LOCAL_CACHE", "1")

import numpy as np
import ml_dtypes

import concourse.bass as bass
import concourse.tile as tile
from concourse import bacc, mybir
from concourse.bass_utils import run_bass_kernel_spmd

AF = mybir.ActivationFunctionType
ALU = mybir.AluOpType
F32 = mybir.dt.float32
BF16 = mybir.dt.float16  # 16-bit compute dtype (fp16: better mantissa than bf16)
AXX = mybir.AxisListType.X

N_CORES = 8
B = 4          # batches per core
N = 2048       # input capsules
O = 64         # output capsules
DI = 16        # input capsule dim
D = 32         # output capsule dim
G = 16         # n-groups of 128 per batch
EPS = 0.5


def build_kernel(nc: bass.Bass, tc: tile.TileContext):
    from contextlib import ExitStack
    ctx = ExitStack()
    x = nc.dram_tensor("x", [B, N, DI], F32, kind="ExternalInput").ap()
    wrep = nc.dram_tensor("wrep", [128, D * DI], F32, kind="ExternalInput").ap()
    m2rep = nc.dram_tensor("m2rep", [128, DI * DI], F32, kind="ExternalInput").ap()
    identb = nc.dram_tensor("identb", [128, 128], BF16, kind="ExternalInput").ap()
    identf = nc.dram_tensor("identf", [128, 128], F32, kind="ExternalInput").ap()
    vout = nc.dram_tensor("vout", [B, O, D], F32, kind="ExternalOutput").ap()

    const = ctx.enter_context(tc.tile_pool(name="const", bufs=1))
    big = ctx.enter_context(tc.tile_pool(name="big", bufs=1))
    small = ctx.enter_context(tc.tile_pool(name="small", bufs=2))
    pbig = ctx.enter_context(tc.tile_pool(name="pbig", bufs=2, space="PSUM"))
    psmall = ctx.enter_context(tc.tile_pool(name="psmall", bufs=1, space="PSUM"))

    # ---- constants ----
    w_sb = const.tile([128, D * DI], F32, tag="w_sb")
    m2_sb = const.tile([128, DI * DI], F32, tag="m2_sb")
    idb = const.tile([128, 128], BF16, tag="idb")
    idf = const.tile([128, 128], F32, tag="idf")
    ones256 = const.tile([128, 4 * O], BF16, tag="ones256")
    blk = [const.tile([128, 4 * O], BF16, tag=f"blk{i}", name=f"blk{i}")
           for i in range(2)]
    wv0f4 = const.tile([128, 2 * DI], F32, tag="wv0f4")
    nc.sync.dma_start(w_sb[:], wrep)
    nc.sync.dma_start(m2_sb[:], m2rep)
    nc.sync.dma_start(idb[:], identb)
    nc.sync.dma_start(idf[:], identf)
    nc.gpsimd.memset(ones256[:], 1.0 / O)
    nc.gpsimd.memset(blk[0][:], 0.0)
    nc.gpsimd.memset(blk[1][:], 0.0)

    # ---- load x:  xr [128, (g, b, i)] ----
    xr = big.tile([128, B * G * DI], F32, tag="xr")
    for b in range(B):
        nc.sync.dma_start(
            xr[:].rearrange("p (g b i) -> p g b i", g=G, b=B)[:, :, b, :],
            x[b].rearrange("(g p) i -> p g i", p=128),
        )

    # ---- squash -> xs_bf [128, (g, b, i)] bf16 ----
    xsq = big.tile([128, B * G * DI], F32, tag="xsq")
    nc.vector.tensor_mul(xsq[:], xr[:], xr[:])
    n2x = small.tile([128, B * G], F32, tag="n2x")
    nc.vector.reduce_sum(n2x[:], xsq[:].rearrange("p (r i) -> p r i", i=DI), axis=AXX)
    lnx = small.tile([128, B * G], F32, tag="lnx")
    nc.scalar.activation(lnx[:], n2x[:], AF.Ln)
    nx = small.tile([128, B * G], F32, tag="nx")
    nc.scalar.activation(nx[:], lnx[:], AF.Exp, scale=0.5)  # sqrt(n2x)
    nc.vector.tensor_scalar_add(nx[:], nx[:], 1e-8)
    denx = small.tile([128, B * G], F32, tag="denx")
    nc.vector.scalar_tensor_tensor(denx[:], n2x[:], EPS, nx[:], op0=ALU.add, op1=ALU.mult)
    nc.vector.reciprocal(denx[:], denx[:])
    gx = small.tile([128, B * G], F32, tag="gx")
    nc.vector.tensor_mul(gx[:], n2x[:], denx[:])
    xs_bf = big.tile([128, B * G * 32], BF16, tag="xs_bf")
    nc.gpsimd.memset(xs_bf[:], 0.0)
    nc.vector.tensor_mul(
        xs_bf[:].rearrange("p (r ii) -> p r ii", ii=32)[:, :, 0:DI],
        xr[:].rearrange("p (r i) -> p r i", i=DI),
        gx[:].unsqueeze(2).broadcast_to([128, B * G, DI]),
    )

    # ---- xsT4 [ (b,i)=64 rows, n=2048 ] via PE transposes ----
    xsT4 = big.tile([128, N], BF16, tag="xsT4")
    for grp in range(8):
        tp = psmall.tile([128, 256], BF16, tag="tp")
        for j in range(2):
            g = grp * 2 + j
            nc.tensor.transpose(
                tp[:, j * 128:(j + 1) * 128],
                xs_bf[:, g * 128:(g + 1) * 128],
                idb[:],
            )
        nc.vector.tensor_copy(xsT4[:, grp * 256:(grp + 1) * 256], tp[:])

    # ---- persistent iter state ----
    e_sb = big.tile([128, B * G * O], BF16, tag="e_sb")
    xz_bf = big.tile([128, B * G * 32], BF16, tag="xz_bf")

    for it in range(3):
        if it > 0:
            # ---- agreements + softmax, 4 strip-groups of 4 ----
            src_blk = blk[it - 1]
            zb = small.tile([128, B * G], F32, tag="zb")
            rz = small.tile([128, B * G], F32, tag="rz")
            for q in range(4):
                aps = pbig.tile([128, 1024], F32, tag="agree")
                for j in range(4):
                    g = 4 * q + j
                    nc.tensor.matmul(
                        aps[:, 256 * j:256 * (j + 1)],
                        lhsT=xsT4[:, 128 * g:128 * (g + 1)],
                        rhs=src_blk[:],
                        start=True,
                        stop=True,
                    )
                nc.scalar.activation(e_sb[:, 1024 * q:1024 * (q + 1)], aps[:], AF.Exp)
                nc.vector.reduce_sum(
                    zb[:, 16 * q:16 * (q + 1)],
                    e_sb[:, 1024 * q:1024 * (q + 1)].rearrange("p (r o) -> p r o", o=O),
                    axis=AXX,
                )
                nc.vector.reciprocal(rz[:, 16 * q:16 * (q + 1)], zb[:, 16 * q:16 * (q + 1)])
                nc.vector.tensor_mul(
                    xz_bf[:, 512 * q:512 * (q + 1)].rearrange("p (r ii) -> p r ii", ii=32),
                    xs_bf[:, 512 * q:512 * (q + 1)].rearrange("p (r ii) -> p r ii", ii=32),
                    rz[:, 16 * q:16 * (q + 1)].unsqueeze(2).broadcast_to([128, 16, 32]),
                )

        # ---- t matmul: tT [ (b,i)=64, (b,o)=256 ] accumulated over 16 strips ----
        stat = xs_bf if it == 0 else xz_bf
        tps = psmall.tile([128, 4 * O], F32, tag="tps")
        for g in range(G):
            nc.tensor.matmul(
                tps[:],
                lhsT=stat[:, 128 * g:128 * (g + 1)],
                rhs=(ones256[:] if it == 0 else e_sb[:, 256 * g:256 * (g + 1)]),
                start=(g == 0),
                stop=(g == G - 1),
            )
        tT = small.tile([128, 4 * O], F32, tag="tT")
        nc.scalar.copy(tT[:], tps[:])

        # ---- diag blocks -> PE transpose -> t4 [128=(b%2,o), (b//2,i)=32] ----
        trn = psmall.tile([128, 2 * 128], F32, tag="trn")
        for half in range(2):
            nc.tensor.transpose(
                trn[:, 128 * half:128 * (half + 1)],
                tT[:, 128 * half:128 * (half + 1)],
                idf[:],
            )
        t4 = small.tile([128, 2 * DI], F32, tag="t4")
        for b in range(B):
            c, bl = b // 2, b % 2
            nc.scalar.copy(
                t4[64 * bl:64 * bl + 64, 16 * c:16 * c + DI],
                trn[64 * bl:64 * bl + 64, 128 * c + 32 * b:128 * c + 32 * b + DI],
            )

        # ---- squash scale h (and q / s) on [o]-partition layout ----
        n24 = small.tile([128, 2], F32, tag="n24")
        if it < 2:
            qm = small.tile([128, 2 * DI * DI], F32, tag="qm")
            nc.vector.tensor_mul(
                qm[:].rearrange("p (c i j) -> p c i j", c=2, j=DI),
                m2_sb[:].rearrange("p (i j) -> p i j", j=DI)
                .unsqueeze(1).broadcast_to([128, 2, DI, DI]),
                t4[:].rearrange("p (c j) -> p c j", j=DI)
                .unsqueeze(2).broadcast_to([128, 2, DI, DI]),
            )
            q4 = small.tile([128, 2 * DI], F32, tag="q4")
            nc.vector.reduce_sum(
                q4[:], qm[:].rearrange("p (r j) -> p r j", j=DI), axis=AXX
            )
            scr = small.tile([128, 2 * DI], F32, tag="scr")
            nc.vector.tensor_mul(scr[:], t4[:], q4[:])
            nc.vector.reduce_sum(
                n24[:], scr[:].rearrange("p (c i) -> p c i", i=DI), axis=AXX
            )
        else:
            sm = small.tile([128, 2 * D * DI], F32, tag="sm")
            nc.vector.tensor_mul(
                sm[:].rearrange("p (c d j) -> p c d j", c=2, j=DI),
                w_sb[:].rearrange("p (d j) -> p d j", j=DI)
                .unsqueeze(1).broadcast_to([128, 2, D, DI]),
                t4[:].rearrange("p (c j) -> p c j", j=DI)
                .unsqueeze(2).broadcast_to([128, 2, D, DI]),
            )
            s4 = small.tile([128, 2 * D], F32, tag="s4")
            nc.vector.reduce_sum(
                s4[:], sm[:].rearrange("p (r j) -> p r j", j=DI), axis=AXX
            )
            scr2 = small.tile([128, 2 * D], F32, tag="scr2")
            nc.vector.tensor_mul(scr2[:], s4[:], s4[:])
            nc.vector.reduce_sum(
                n24[:], scr2[:].rearrange("p (c d) -> p c d", d=D), axis=AXX
            )
        lnt = small.tile([128, 2], F32, tag="lnt")
        nc.scalar.activation(lnt[:], n24[:], AF.Ln)
        nt = small.tile([128, 2], F32, tag="nt")
        nc.scalar.activation(nt[:], lnt[:], AF.Exp, scale=0.5)  # sqrt(n24)
        nc.vector.tensor_scalar_add(nt[:], nt[:], 1e-8)
        dent = small.tile([128, 2], F32, tag="dent")
        nc.vector.scalar_tensor_tensor(dent[:], n24[:], EPS, nt[:], op0=ALU.add, op1=ALU.mult)
        nc.vector.reciprocal(dent[:], dent[:])
        h4 = small.tile([128, 2], F32, tag="h4")
        nc.vector.tensor_mul(h4[:], n24[:], dent[:])

        if it < 2:
            # ---- wv = h*q (+ wv0); PE transpose -> blockdiag rhs ----
            if it == 0:
                wvq = wv0f4
                nc.vector.tensor_mul(
                    wvq[:].rearrange("p (c i) -> p c i", i=DI),
                    q4[:].rearrange("p (c i) -> p c i", i=DI),
                    h4[:].unsqueeze(2).broadcast_to([128, 2, DI]),
                )
            else:
                wvq = small.tile([128, 2 * DI], F32, tag="wvq")
                nc.vector.tensor_mul(
                    wvq[:].rearrange("p (c i) -> p c i", i=DI),
                    q4[:].rearrange("p (c i) -> p c i", i=DI),
                    h4[:].unsqueeze(2).broadcast_to([128, 2, DI]),
                )
                nc.vector.tensor_add(wvq[:], wvq[:], wv0f4[:])
            wv4p = small.tile([128, 128], F32, tag="wv4p")
            for b in range(B):
                c, bl = b // 2, b % 2
                nc.gpsimd.tensor_copy(
                    wv4p[64 * bl:64 * bl + 64, 32 * b:32 * b + DI],
                    wvq[64 * bl:64 * bl + 64, 16 * c:16 * c + DI],
                )
            wvt_ps = psmall.tile([128, 128], F32, tag="wvt")
            nc.tensor.transpose(wvt_ps[:], wv4p[:], idf[:])
            for b in range(B):
                bl = b % 2
                nc.scalar.copy(
                    blk[it][32 * b:32 * b + 16, 64 * b:64 * b + 64],
                    wvt_ps[32 * b:32 * b + 16, 64 * bl:64 * bl + 64],
                )
        else:
            # ---- output v = h * s ----
            v4 = small.tile([128, 2 * D], F32, tag="v4")
            nc.vector.tensor_mul(
                v4[:].rearrange("p (c d) -> p c d", d=D),
                s4[:].rearrange("p (c d) -> p c d", d=D),
                h4[:].unsqueeze(2).broadcast_to([128, 2, D]),
            )
            for b in range(B):
                nc.sync.dma_start(
                    vout[b],
                    v4[64 * (b % 2):64 * (b % 2) + 64, 32 * (b // 2):32 * (b // 2) + 32],
                )
    ctx.close()


_CACHE = {}


def _get_module():
    if "nc" not in _CACHE:
        nc = bacc.Bacc("TRN2", target_bir_lowering=False, debug=False,
                       enable_asserts=False, num_devices=N_CORES)
        with tile.TileContext(nc) as tc:
            build_kernel(nc, tc)
        nc.compile()
        _CACHE["nc"] = nc
    return _CACHE["nc"]


def _host_inputs(input_vectors, weight_matrix):
    W0 = np.asarray(weight_matrix, dtype=np.float32)[0]          # [O, D, DI]
    M2 = np.einsum("odi,odj->oij", W0, W0).astype(np.float32)    # [O, DI, DI]
    wrep = np.tile(W0.reshape(O, D * DI), (2, 1)).astype(np.float32)
    m2rep = np.tile(M2.reshape(O, DI * DI), (2, 1)).astype(np.float32)
    identb = np.eye(128, dtype=np.float16)
    identf = np.eye(128, dtype=np.float32)
    x = np.ascontiguousarray(np.asarray(input_vectors, dtype=np.float32))
    in_maps = []
    for c in range(N_CORES):
        in_maps.append({
            "x": np.ascontiguousarray(x[c * B:(c + 1) * B]),
            "wrep": wrep,
            "m2rep": m2rep,
            "identb": identb,
            "identf": identf,
        })
    return in_maps


def run(input_vectors, weight_matrix, trace=False, tmpdir=None):
    nc = _get_module()
    in_maps = _host_inputs(input_vectors, weight_matrix)
    res = run_bass_kernel_spmd(
        nc, in_maps, core_ids=list(range(N_CORES)), trace=trace, tmpdir=tmpdir
    )
    out = np.concatenate([res.results[c]["vout"] for c in range(N_CORES)], axis=0)
    return out.astype(np.float32), res


def kernel(input_vectors, weight_matrix):
    out, _ = run(input_vectors, weight_matrix, trace=False)
    return out
